# revision 18
# baseline (speedup 1.0000x reference)
"""Trainium2 Bass kernel for nn_AttentionLayer (segment softmax attention pooling).

Computation (reference):
    h = tanh(x @ W1 + b1)            # [N, A]
    s = h @ W2 + b2                  # [N, 1]
    per-segment softmax over s, out[b] = sum_i softmax_w_i * x_i   # [B, D]

v6 strategy (per core, N/8 = 62500 rows, all device streams in bf16):
  - Host pre-transposes x to xT [D, rows], bf16 (tolerance is 2e-2; bf16
    costs ~0.4%).  Loads are region-batched (10 chunks per DMA) to keep
    the SP sequencer's per-DMA issue cost off the critical path.
  - PE pass 1: hraw = W1^T @ xT per 1536-row chunk (W1 stationary).
  - ACT: th = tanh(hraw + b1) -> SBUF bf16.  The hard floor: 8M
    lane-elements = ~64us at 1.2 GHz; everything else hides under it.
  - PE pass 2: scores via one-hot W2 stationaries, accumulating [1, 512]
    rows into a compact [32, 512] PSUM region (30 rows = 10 chunks), so
    exp is ~100x cheaper than a broadcast exp (ACT cost ~ free size).
  - ACT: e = exp(s + b2) -> ebatch [30, 512] bf16.
  - XBAR DMA transposes (SBUF->SBUF, 16x128-block granularity):
      * xT region [128, 15360] -> xnat [128, 120, 128]: per-128-column
        block transpose = x rows on partitions (natural orientation).
      * ebatch [32, 512] -> ecmt [128, 4, 32]: e-columns per 128-row tile.
  - PE pass 3 (weighted sum): per 128-row tile, matmul with the e-column
    as a 1-column stationary and the xnat tile as moving:
        out[1, D] = sum_rows e[row] * x[row, :]
    accumulated over a chunk's 12 tiles into a [1, 128] PSUM slot carved
    out of the dead score region (4 base partitions x 4 column slots).
    No partition broadcast, no GpSimd, no DVE multiply.
  - DVE only copies the per-chunk sums PSUM->SBUF (15 small copies).
  - Segment logic on host, as the baseline: windows fully inside one
    segment use device sums; boundary windows are recomputed on host
    from x and the exported e.  Denominators via bincount over e.  exp
    without max-subtraction is safe (|s| < ~4) and numerator/denominator
    use identical e values.
"""

import numpy as np
import ml_dtypes

# Problem constants (hardcoded per contract; kernel.py must be self-contained).
N = 500_000
D = 128
A = 128
B = 256
NCORES = 8
RPC = N // NCORES            # rows per core = 62500
CHUNK = 1536                 # rows per chunk (3 PSUM banks in f32)
TPC = CHUNK // 128           # 128-row tiles per chunk = 12
SUBW = 512                   # score sub-matmul moving width
SUBJ = CHUNK // SUBW         # score sub-matmuls per chunk = 3
NCHUNK = -(-RPC // CHUNK)    # 41
RPAD = NCHUNK * CHUNK        # 62976
REGC = 10                    # chunks per region (30 of 32 score rows)
REG_ROWS = REGC * SUBJ       # 30
RTILES = REGC * TPC          # 120 row-tiles per region
NREG = -(-NCHUNK // REGC)    # 5 (last region holds 1 chunk)
EROWS = NCHUNK * SUBJ        # valid eout rows = 123
MM_N = 512                   # matmul moving-width limit

_prog_cache = {}


def _build_program(b2val: float):
    import concourse.bacc as bacc
    from concourse import mybir
    from concourse.tile import TileContext

    f32 = mybir.dt.float32
    bf16 = mybir.dt.bfloat16
    nc = bacc.Bacc("TRN2", target_bir_lowering=False, debug=False,
                   num_devices=NCORES)

    xt = nc.dram_tensor("xt", [D, NREG * REGC * CHUNK], bf16,
                        kind="ExternalInput")
    xnt = nc.dram_tensor("xnt", [128, NREG * RTILES * 128], bf16,
                         kind="ExternalInput")
    w1 = nc.dram_tensor("w1", [D, A], bf16, kind="ExternalInput")
    w2oh = nc.dram_tensor("w2oh", [A, REG_ROWS * 32], bf16,
                          kind="ExternalInput")
    b1 = nc.dram_tensor("b1", [A, 1], f32, kind="ExternalInput")
    b2 = nc.dram_tensor("b2", [128, 1], f32, kind="ExternalInput")
    wacc = nc.dram_tensor("wacc", [NCHUNK, D], f32, kind="ExternalOutput")
    eout = nc.dram_tensor("eout", [NREG * REG_ROWS, SUBW], bf16,
                          kind="ExternalOutput")

    with TileContext(nc) as tc:
        with tc.tile_pool(name="const", bufs=1) as cpool, \
             tc.tile_pool(name="xrp", bufs=4) as xrpool, \
             tc.tile_pool(name="xnp", bufs=8) as xnpool, \
             tc.tile_pool(name="thp", bufs=3) as thpool, \
             tc.tile_pool(name="ebp", bufs=2) as ebpool, \
             tc.tile_pool(name="etp", bufs=2) as etpool, \
             tc.tile_pool(name="accp", bufs=2) as apool, \
             tc.tile_pool(name="psh", bufs=2, space="PSUM") as psh, \
             tc.tile_pool(name="pss", bufs=2, space="PSUM") as pss:

            w1sb = cpool.tile([D, A], bf16, tag="w1")
            w2sb = cpool.tile([A, REG_ROWS * 32], bf16, tag="w2")
            b1sb = cpool.tile([A, 1], f32, tag="b1")
            b2sb = cpool.tile([128, 1], f32, tag="b2")
            nc.sync.dma_start(out=w1sb[:], in_=w1[:])
            nc.sync.dma_start(out=w2sb[:], in_=w2oh[:])
            nc.sync.dma_start(out=b1sb[:], in_=b1[:])
            nc.sync.dma_start(out=b2sb[:], in_=b2[:])


            xregs, xnats, ths, hregs = {}, {}, {}, {}
            sregs, ebatches, ecmts = {}, {}, {}

            QC = 4
            NQUAD = (NCHUNK + QC - 1) // QC

            def load_quad(q):
                if q >= NQUAD:
                    return
                nch = min(QC * CHUNK, (NCHUNK - QC * q) * CHUNK)
                xr = xrpool.tile([D, QC * CHUNK], bf16, tag="xr", name="xr")
                base = q * QC * CHUNK
                if q == 0:
                    for i in range(QC):
                        nc.sync.dma_start(
                            out=xr[:, i * CHUNK:(i + 1) * CHUNK],
                            in_=xt[:, base + i * CHUNK:
                                   base + (i + 1) * CHUNK])
                else:
                    nc.gpsimd.dma_start(out=xr[:, 0:nch],
                                        in_=xt[:, base:base + nch])
                xregs[q] = xr
                # natural-orientation copy, host-pretiled: [p, t, d] with
                # element (p, t, d) = x[t*128 + p, d]
                ntl = nch // 128
                xn = xnpool.tile([128, QC * TPC, 128], bf16, tag="xn",
                                 name="xn")
                nc.sync.dma_start(
                    out=xn[:, 0:ntl, :],
                    in_=xnt[:, q * QC * TPC * 128:q * QC * TPC * 128
                            + ntl * 128])
                xnats[q] = xn

            def hmm(c):
                hreg = psh.tile([128, CHUNK], f32, tag="hreg")
                hregs[c] = hreg
                xr = xregs[c // QC]
                off = (c % QC) * CHUNK
                for i in range(SUBJ):
                    nc.tensor.matmul(
                        out=hreg[:, i * MM_N:(i + 1) * MM_N],
                        lhsT=w1sb[:],
                        rhs=xr[:, off + i * MM_N:off + (i + 1) * MM_N],
                        start=True, stop=True)
                if c % QC == QC - 1 or c == NCHUNK - 1:
                    xregs.pop(c // QC)

            def tanh(c):
                th = thpool.tile([A, CHUNK], bf16, tag="th")
                ths[c] = th
                nc.scalar.activation(
                    out=th[:], in_=hregs.pop(c),
                    func=mybir.ActivationFunctionType.Tanh,
                    bias=b1sb[:, 0:1])

            def submms(c):
                g, cc = c // REGC, c % REGC
                if cc == 0:
                    sregs[g] = pss.tile([128, SUBW], f32, tag="sb",
                                        name="sb")
                sb = sregs[g]
                th = ths.pop(c)
                last_c = min((g + 1) * REGC, NCHUNK) - 1
                for j in range(SUBJ):
                    r = cc * SUBJ + j
                    # one-hot stationary drops scores on region row r; the
                    # whole [32, SUBW] block accumulates across the region.
                    nc.tensor.matmul(out=sb[0:32, :],
                                     lhsT=w2sb[:, r * 32:(r + 1) * 32],
                                     rhs=th[:, j * SUBW:(j + 1) * SUBW],
                                     start=(r == 0),
                                     stop=(c == last_c and j == SUBJ - 1),
                                     skip_group_check=True)

            def exp_region(g):
                lo = g * REGC
                hi = min(lo + REGC, NCHUNK)
                rows = (hi - lo) * SUBJ
                eb = ebpool.tile([32, SUBW], bf16, tag="eb")
                ebatches[g] = eb
                nc.scalar.activation(
                    out=eb[0:rows, :], in_=sregs[g][0:rows, :],
                    func=mybir.ActivationFunctionType.Exp,
                    bias=b2sb[0:rows, 0:1])
                nc.scalar.dma_start(
                    out=eout[g * REG_ROWS:g * REG_ROWS + rows, :],
                    in_=eb[0:rows, :])
                # e-columns per 128-row tile: column t lives at
                # ecmt[:, t % 4, t // 4]
                et = etpool.tile([128, 4, 32], bf16, tag="et", name="et")
                nc.scalar.dma_start_transpose(out=et[:], in_=eb[0:32, :])
                ecmts[g] = et

            def wsum_chunk(c):
                g, cc = c // REGC, c % REGC
                sb, et = sregs[g], ecmts[g]
                xn = xnats[c // QC]
                p0 = (cc % 3) * 32
                k0 = (cc // 3) * 128
                for t12 in range(TPC):
                    tg = cc * TPC + t12
                    tl = (c % QC) * TPC + t12
                    nc.tensor.matmul(
                        out=sb[p0:p0 + 1, k0:k0 + 128],
                        lhsT=et[:, tg % 4:tg % 4 + 1, tg // 4:tg // 4 + 1],
                        rhs=xn[:, tl, :],
                        start=(t12 == 0), stop=(t12 == TPC - 1),
                        skip_group_check=True)
                if c % QC == QC - 1 or c == NCHUNK - 1:
                    xnats.pop(c // QC)

            def wsum_export(g):
                lo = g * REGC
                hi = min(lo + REGC, NCHUNK)
                sb = sregs.pop(g)
                wr = apool.tile([1, REGC * D], f32, tag="wr", name="wr")
                for cc in range(hi - lo):
                    p0 = (cc % 3) * 32
                    k0 = (cc // 3) * 128
                    nc.vector.tensor_copy(
                        out=wr[0:1, cc * D:(cc + 1) * D],
                        in_=sb[p0:p0 + 1, k0:k0 + 128])
                nc.scalar.dma_start(out=wacc[lo:hi, :],
                                  in_=wr[0:1, 0:(hi - lo) * D])
                ebatches.pop(g)
                ecmts.pop(g)

            load_quad(0)
            load_quad(1)
            hmm(0)
            done_w = 0
            for c in range(NCHUNK):
                if c % QC == 0:
                    load_quad(c // QC + 2)
                if c + 1 < NCHUNK:
                    hmm(c + 1)
                tanh(c)
                submms(c)
                # region g's exp is emitted one chunk after it closes (so it
                # never sits at the ACT queue head in front of a tanh); its
                # weighted sums then trickle one chunk per iteration.
                if c >= 1 and (c - 1) % REGC == REGC - 1:
                    exp_region((c - 1) // REGC)
                if done_w < NCHUNK and done_w // REGC in ecmts:
                    wsum_chunk(done_w)
                    done_w += 1
                    if done_w % REGC == 0:
                        wsum_export(done_w // REGC - 1)
            exp_region(NREG - 1)
            while done_w < NCHUNK:
                wsum_chunk(done_w)
                done_w += 1
                if done_w % REGC == 0:
                    wsum_export(done_w // REGC - 1)
            wsum_export(NREG - 1)

    nc.compile()
    return nc


def _run_device(xt_full, xn_full, W1, W2, b1, b2, trace=False):
    from concourse.bass_utils import run_bass_kernel_spmd

    key = float(b2)
    if key not in _prog_cache:
        _prog_cache[key] = _build_program(key)
    nc = _prog_cache[key]

    bf16 = ml_dtypes.bfloat16
    w1_in = np.ascontiguousarray(W1.astype(bf16))
    w2oh = np.zeros((A, REG_ROWS, 32), dtype=np.float32)
    for r in range(REG_ROWS):
        w2oh[:, r, r % 32] = W2.reshape(-1)
    w2_in = np.ascontiguousarray(w2oh.reshape(A, REG_ROWS * 32).astype(bf16))
    b1_in = np.ascontiguousarray(b1.reshape(A, 1), dtype=np.float32)
    b2_in = np.full((128, 1), np.float32(b2), dtype=np.float32)

    in_maps = [{"xt": xt_full[i], "xnt": xn_full[i], "w1": w1_in,
                "w2oh": w2_in, "b1": b1_in, "b2": b2_in}
               for i in range(NCORES)]
    res = run_bass_kernel_spmd(nc, in_maps, core_ids=list(range(NCORES)),
                               trace=trace)
    return res


def kernel(x, batch_index, W1, b1, W2, b2, _want_results=False, _trace=False):
    x = np.ascontiguousarray(np.asarray(x, dtype=np.float32))
    bi64 = np.asarray(batch_index).astype(np.int64)
    W1 = np.asarray(W1, dtype=np.float32)
    b1 = np.asarray(b1, dtype=np.float32)
    W2 = np.asarray(W2, dtype=np.float32)
    b2v = float(np.asarray(b2, dtype=np.float32).reshape(-1)[0])

    assert x.shape == (N, D)

    # Host prep: xT [D, rows] bf16 shards for the score path, plus a
    # natural-orientation pre-tiled copy xnt[p, t, d] = x[t*128 + p, d]
    # so the weighted-sum matmuls read full-rate contiguous DMA lines.
    bf16 = ml_dtypes.bfloat16
    PADR = NREG * REGC * CHUNK
    xbf = x.astype(bf16)
    xtf = np.ascontiguousarray(xbf.T)
    xt_shards, xn_shards = [], []
    for i in range(NCORES):
        sh = np.zeros((D, PADR), dtype=bf16)
        sh[:, :RPC] = xtf[:, i * RPC:(i + 1) * RPC]
        xt_shards.append(sh)
        nat = np.zeros((PADR, D), dtype=bf16)
        nat[:RPC] = xbf[i * RPC:(i + 1) * RPC]
        tiled = np.ascontiguousarray(
            nat.reshape(PADR // 128, 128, D).transpose(1, 0, 2)
        ).reshape(128, PADR // 128 * D)
        xn_shards.append(tiled)

    res = _run_device(xt_shards, xn_shards, W1, W2, b1, b2v, trace=_trace)

    # Gather device outputs.  eout rows are (chunk, sub)-major so a plain
    # reshape recovers instance order.
    e = np.empty(N, dtype=np.float32)
    waccs = []
    for i in range(NCORES):
        eo = np.asarray(res.results[i]["eout"])[:EROWS]
        e[i * RPC:(i + 1) * RPC] = \
            eo.astype(np.float32).reshape(-1)[:RPC]
        waccs.append(np.asarray(res.results[i]["wacc"]))

    # Denominators: segment sums of e (same values the device used).
    denom = np.bincount(bi64, weights=e.astype(np.float64), minlength=B)

    # Numerators: pure windows from device sums; boundary windows recomputed.
    WIN = CHUNK
    num = np.zeros((B, D), dtype=np.float64)
    for i in range(NCORES):
        wacc_i = waccs[i]
        base = i * RPC
        for w in range(NCHUNK):
            glo = base + w * WIN
            if glo >= base + RPC:
                break
            ghi = min(glo + WIN, base + RPC)
            b_first = bi64[glo]
            b_last = bi64[ghi - 1]
            if b_first == b_last:
                # Window entirely in one segment (zero-pad rows contribute 0).
                num[b_first] += wacc_i[w, :]
            else:
                sub = bi64[glo:ghi]
                cuts = np.flatnonzero(np.diff(sub)) + 1
                bounds = np.concatenate(([0], cuts, [ghi - glo]))
                for k in range(len(bounds) - 1):
                    lo, hi = glo + bounds[k], glo + bounds[k + 1]
                    num[sub[bounds[k]]] += \
                        e[lo:hi].astype(np.float64) @ x[lo:hi].astype(np.float64)

    dn = denom[:, None]
    out = np.divide(num, dn, out=np.zeros_like(num), where=dn > 0)
    out = out.astype(np.float32)
    if _want_results:
        return out, res
    return out


# revision 19
# speedup vs baseline: 1.1365x; 1.1365x over previous
"""Trainium2 Bass kernel for nn_AttentionLayer (segment softmax attention pooling).

Computation (reference):
    h = tanh(x @ W1 + b1)            # [N, A]
    s = h @ W2 + b2                  # [N, 1]
    per-segment softmax over s, out[b] = sum_i softmax_w_i * x_i   # [B, D]

Strategy:
  - Shard the N=500k instances across 8 NeuronCores (data parallel), weights
    replicated. Host pre-transposes x so each core streams xT [D=128, rows]
    tiles with fully contiguous DMA and D on partitions.
  - Per core, one pass over x (f32r matmuls, software-pipelined chunks):
      PE:  hT = W1^T @ xT            (W1 stationary, rows stream)
      ACT: th = tanh(hT + b1)
      PE:  sbc = W2rep^T @ th        (score broadcast to all 128 partitions)
      ACT: ebc = exp(sbc + b2)
      DVE: affine_mul_reduce(xT * ebc) summed per fixed 1024-row window
  - Device outputs: per-window weighted sums WACC [D, nwin] and the raw e row
    (e values for every instance).  All segment logic is on the host: window
    sums for windows fully inside one segment are used directly; windows that
    contain a segment boundary are recomputed on the host from x and the
    exported e (a few hundred small dot products).  Denominators come from
    bincount over the exported e.  exp() without max-subtraction is safe here
    (scores are O(+-5)), and numerator/denominator use identical e values.
"""

import numpy as np

# Problem constants (hardcoded per contract; kernel.py must be self-contained).
N = 500_000
D = 128
A = 128
B = 256
NCORES = 8
RPC = N // NCORES            # rows per core = 62500
CHUNK = 2048                 # rows per streamed tile
WIN = 1024                   # rows per reduction window
NCHUNK = -(-RPC // CHUNK)    # 31
RPAD = NCHUNK * CHUNK        # 63488
NWIN = RPAD // WIN           # 62
MM_N = 512                   # fp32 moving-operand max free dim

_prog_cache = {}


def _build_program(b2val: float):
    import concourse.bacc as bacc
    from concourse import mybir
    from concourse.tile import TileContext

    f32 = mybir.dt.float32
    f32r = mybir.dt.float32r
    nc = bacc.Bacc("TRN2", target_bir_lowering=False, debug=False,
                   num_devices=NCORES)

    # Declared f32r so the load is a plain (cast-free, HWDGE-eligible) copy
    # and the matmul consumer passes BIR verification; the bytes are ordinary
    # fp32 (np float32 maps to both), and the reduce path bitcasts back to
    # f32 so the numerator sees full-precision x.
    xt = nc.dram_tensor("xt", [D, RPAD], f32r, kind="ExternalInput")
    w1 = nc.dram_tensor("w1", [D, A], f32, kind="ExternalInput")
    w2r = nc.dram_tensor("w2r", [A, 128], f32, kind="ExternalInput")
    b1 = nc.dram_tensor("b1", [A, 1], f32, kind="ExternalInput")
    b2 = nc.dram_tensor("b2", [128, 1], f32, kind="ExternalInput")
    wacc = nc.dram_tensor("wacc", [D, NWIN], f32, kind="ExternalOutput")
    eout = nc.dram_tensor("eout", [NCHUNK, CHUNK], f32, kind="ExternalOutput")

    with TileContext(nc) as tc:
        with tc.tile_pool(name="const", bufs=1) as cpool, \
             tc.tile_pool(name="xtp", bufs=5) as xpool, \
             tc.tile_pool(name="thp", bufs=2) as thpool, \
             tc.tile_pool(name="ebp", bufs=3) as ebpool, \
             tc.tile_pool(name="junkp", bufs=2) as jpool, \
             tc.tile_pool(name="accp", bufs=1) as apool, \
             tc.tile_pool(name="psb", bufs=1, space="PSUM") as psb:

            # f32r (reduced-mantissa fp32) runs the PE at ~1 cycle/row vs 4
            # for fp32; all f32r tensors are declared f32r in DRAM so loads
            # are plain cast-free copies.
            w1sb = cpool.tile([D, A], f32r, tag="w1")
            w2rsb = cpool.tile([A, 128], f32r, tag="w2r")
            b1sb = cpool.tile([A, 1], f32, tag="b1")
            b2sb = cpool.tile([128, 1], f32, tag="b2")
            nc.gpsimd.dma_start(out=w1sb[:], in_=w1[:])
            nc.gpsimd.dma_start(out=w2rsb[:], in_=w2r[:])
            nc.sync.dma_start(out=b1sb[:], in_=b1[:])
            nc.sync.dma_start(out=b2sb[:], in_=b2[:])

            waccsb = apool.tile([D, NWIN], f32, tag="wacc")
            nc.vector.memset(waccsb[:], 0.0)

            # One PSUM tensor spanning all 8 banks: lower half holds hT
            # (pre-tanh), upper half holds the broadcast scores (pre-exp).
            # Tile tracks deps bank-granular, so both activations read a
            # full 2048-wide region in a single op.
            pbig = psb.tile([128, 2 * CHUNK], f32, tag="pbig")
            hreg = pbig[:, 0:CHUNK]
            sreg = pbig[:, CHUNK:2 * CHUNK]

            # Software pipeline: stage 1 (load + h-matmuls + tanh) for chunk c
            # is emitted in the same iteration as stage 2 (score matmuls +
            # exp + reduce) for chunk c-1, so the scheduler's program-order
            # priority keeps every engine fed: PE prefers next-chunk h-matmuls
            # over current-chunk score-matmuls, which keeps ACT gap-free.
            HALF = CHUNK // 2
            xtiles, ths = {}, {}

            def load_chunk(c, split=False):
                xtile = xpool.tile([D, CHUNK], f32r, tag="x")
                base = c * CHUNK
                if split:
                    # First chunk: two half-loads so the first tanh's matmuls
                    # unblock after 512 KB instead of 1 MB.
                    nc.gpsimd.dma_start(out=xtile[:, :HALF],
                                        in_=xt[:, base:base + HALF])
                    nc.gpsimd.dma_start(out=xtile[:, HALF:],
                                        in_=xt[:, base + HALF:base + CHUNK])
                else:
                    nc.gpsimd.dma_start(out=xtile[:],
                                        in_=xt[:, base:base + CHUNK])
                xtiles[c] = xtile

            load_chunk(0, split=True)
            if NCHUNK > 1:
                load_chunk(1)
            for c in range(NCHUNK + 1):
                if c + 2 < NCHUNK:
                    load_chunk(c + 2)
                if c < NCHUNK:
                    xtile = xtiles[c]
                    th = thpool.tile([A, CHUNK], f32r, tag="th")
                    ths[c] = th
                    for i in range(CHUNK // MM_N):
                        nc.tensor.matmul(
                            out=hreg[:, i * MM_N:(i + 1) * MM_N],
                            lhsT=w1sb[:],
                            rhs=xtile[:, i * MM_N:(i + 1) * MM_N],
                            start=True, stop=True)
                    nc.scalar.activation(
                        out=th[:],
                        in_=hreg,
                        func=mybir.ActivationFunctionType.Tanh,
                        bias=b1sb[:, 0:1])
                if c >= 1:
                    p = c - 1
                    xtile_p, th_p = xtiles.pop(p), ths.pop(p)
                    for i in range(CHUNK // MM_N):
                        nc.tensor.matmul(out=sreg[:, i * MM_N:(i + 1) * MM_N],
                                         lhsT=w2rsb[:],
                                         rhs=th_p[:, i * MM_N:(i + 1) * MM_N],
                                         start=True, stop=True)
                    eb = ebpool.tile([128, CHUNK], f32, tag="eb")
                    nc.scalar.activation(out=eb[:], in_=sreg,
                                         func=mybir.ActivationFunctionType.Exp,
                                         bias=b2sb[:, 0:1])
                    for w in range(CHUNK // WIN):
                        gw = p * (CHUNK // WIN) + w
                        junk = jpool.tile([D, 1], f32, tag="junk")
                        nc.vector.affine_mul_reduce(
                            out=junk[:].to_broadcast([D, WIN]),
                            accum_out=waccsb[:, gw:gw + 1],
                            in0=xtile_p[:, w * WIN:(w + 1) * WIN].bitcast(f32),
                            in1=eb[:, w * WIN:(w + 1) * WIN],
                            scale=1.0,
                            bias=0.0)
                    nc.sync.dma_start(out=eout[p:p + 1, :], in_=eb[0:1, :])

            nc.sync.dma_start(out=wacc[:], in_=waccsb[:])

    nc.compile()
    return nc


def _run_device(xt_shards, W1, W2, b1, b2, trace=False):
    from concourse.bass_utils import run_bass_kernel_spmd

    key = float(b2)
    if key not in _prog_cache:
        _prog_cache[key] = _build_program(key)
    nc = _prog_cache[key]

    w1_in = np.ascontiguousarray(W1, dtype=np.float32)
    w2r_in = np.ascontiguousarray(np.tile(W2.reshape(A, 1), (1, 128)),
                                  dtype=np.float32)
    b1_in = np.ascontiguousarray(b1.reshape(A, 1), dtype=np.float32)
    b2_in = np.full((128, 1), np.float32(b2), dtype=np.float32)

    in_maps = [{"xt": xt_shards[i], "w1": w1_in, "w2r": w2r_in, "b1": b1_in,
                "b2": b2_in}
               for i in range(NCORES)]
    res = run_bass_kernel_spmd(nc, in_maps, core_ids=list(range(NCORES)),
                               trace=trace)
    return res


def kernel(x, batch_index, W1, b1, W2, b2, _want_results=False, _trace=False):
    x = np.ascontiguousarray(np.asarray(x, dtype=np.float32))
    bi = np.asarray(batch_index)
    bi64 = bi.astype(np.int64)
    W1 = np.asarray(W1, dtype=np.float32)
    b1 = np.asarray(b1, dtype=np.float32)
    W2 = np.asarray(W2, dtype=np.float32)
    b2v = float(np.asarray(b2, dtype=np.float32).reshape(-1)[0])

    assert x.shape == (N, D)

    # Host pre-transpose: xT [D, N], then per-core zero-padded shards.
    xtf = np.ascontiguousarray(x.T)
    xt_shards = []
    for i in range(NCORES):
        sh = np.zeros((D, RPAD), dtype=np.float32)
        sh[:, :RPC] = xtf[:, i * RPC:(i + 1) * RPC]
        xt_shards.append(sh)

    res = _run_device(xt_shards, W1, W2, b1, b2v, trace=_trace)

    # Gather device outputs.
    e = np.empty(N, dtype=np.float32)
    waccs = []
    for i in range(NCORES):
        e[i * RPC:(i + 1) * RPC] = \
            res.results[i]["eout"].reshape(-1)[:RPC]
        waccs.append(res.results[i]["wacc"])

    # Denominators: segment sums of e (same values the device used).
    denom = np.bincount(bi64, weights=e.astype(np.float64), minlength=B)

    # Numerators: pure windows from device sums; boundary windows recomputed.
    num = np.zeros((B, D), dtype=np.float64)
    for i in range(NCORES):
        wacc_i = waccs[i]
        base = i * RPC
        for w in range(NWIN):
            glo = base + w * WIN
            if glo >= base + RPC:
                break
            ghi = min(glo + WIN, base + RPC)
            b_first = bi64[glo]
            b_last = bi64[ghi - 1]
            if b_first == b_last:
                # Window entirely in one segment (zero-pad rows contribute 0).
                num[b_first] += wacc_i[:, w]
            else:
                sub = bi64[glo:ghi]
                cuts = np.flatnonzero(np.diff(sub)) + 1
                bounds = np.concatenate(([0], cuts, [ghi - glo]))
                for k in range(len(bounds) - 1):
                    lo, hi = glo + bounds[k], glo + bounds[k + 1]
                    num[sub[bounds[k]]] += \
                        e[lo:hi].astype(np.float64) @ x[lo:hi].astype(np.float64)

    dn = denom[:, None]
    out = np.divide(num, dn, out=np.zeros_like(num), where=dn > 0)
    out = out.astype(np.float32)
    if _want_results:
        return out, res
    return out

